# revision 12
# baseline (speedup 1.0000x reference)
"""Trainium2 Bass kernel for VITS-style multi-head attention with windowed
relative position embeddings (window=4), batch 8 x channels 512 x time 1024.

Strategy: pure data parallelism — one batch element per NeuronCore (8 cores).
Per core:
  - Q/K projections in [channel, time] layout (float32r); V projected directly
    transposed as [time, channel] in bf16 (PV stationary operand).
  - Per head: the relative-key logits R9 = Qs @ Krel^T [T, 9] are computed for
    all 8 row-tiles up-front (prefetched one head ahead) and skewed into
    banded [128, 8*136] bias rows via one stride-137-write / stride-136-read
    DRAM round trip (writing row p at pitch 137 and reading at pitch 136
    shifts row p right by p).
  - Per 128-row tile: scores = Qs K^T via PE (f32r), banded bias added in
    PSUM, exp (ACT, free-dim accumulate, no max subtraction — scores ~N(0,1)),
    normalize (DVE, bf16 out), PE-transpose P (bf16) for the PV matmul.
  - Value-side band P9[i,d] = P[i, i+d-4] extracted per pair of row tiles via
    the inverse skew (write pitch 136, read pitch 137) and folded into the
    output PSUM as a [9,64] matmul alongside PV.
"""

import os
import sys

sys.path.insert(0, "/opt/trn_rl_repo")

import numpy as np

import concourse.bass as bass
import concourse.mybir as mybir
import concourse.tile as tile
from concourse import bacc
from concourse.bass_utils import run_bass_kernel_spmd
from concourse.masks import make_identity

B, C, T = 8, 512, 1024
H = 8
KC = C // H  # 64
WINDOW = 4
M_REL = 2 * WINDOW + 1  # 9
SCALE = 1.0 / float(np.sqrt(KC))
N_CORES = 8
NT = T // 128  # 8 row tiles per head
SEG = 128 * 137  # per-row-tile skew scratch segment (elements)

F32 = mybir.dt.float32
F32R = mybir.dt.float32r
BF16 = mybir.dt.bfloat16
AF = mybir.ActivationFunctionType


def r(ap):
    return ap.bitcast(F32R)


def build_kernel():
    nc = bacc.Bacc(
        "TRN2", target_bir_lowering=False, debug=False, num_devices=N_CORES
    )

    x_d = nc.dram_tensor("x", [C, T], F32, kind="ExternalInput")
    c_d = nc.dram_tensor("c", [C, T], F32, kind="ExternalInput")
    w_d = {
        n: nc.dram_tensor(n, [C, C], F32, kind="ExternalInput")
        for n in ("Wq", "Wk", "Wv", "Wo")
    }
    b_d = {
        n: nc.dram_tensor(n, [C], F32, kind="ExternalInput")
        for n in ("bq", "bk", "bv", "bo")
    }
    ek_d = nc.dram_tensor("emb_rel_k", [1, M_REL, KC], F32, kind="ExternalInput")
    ev_d = nc.dram_tensor("emb_rel_v", [1, M_REL, KC], F32, kind="ExternalInput")
    y_d = nc.dram_tensor("y", [C, T], F32, kind="ExternalOutput")

    # DRAM skew scratches, one 128x137-element segment per row tile.
    # band (f32): write [128,9] at pitch 137, read [128,136] at pitch 136
    # (gaps must stay zero). extr (bf16): write [128,136] at pitch 136, read
    # [128,9] at pitch 137.
    band_scr = [
        nc.dram_tensor(f"band_scr{i}", [NT, SEG], F32, kind="Internal")
        for i in range(2)
    ]
    extr_scr = [
        nc.dram_tensor(f"extr_scr{i}", [NT, SEG], BF16, kind="Internal")
        for i in range(2)
    ]

    CC = C // 128  # 4 channel chunks

    with tile.TileContext(nc) as tc:
        with (
            tc.tile_pool(name="const", bufs=1) as constp,
            tc.tile_pool(name="wpool", bufs=1) as wpool,
            tc.tile_pool(name="xc", bufs=1) as xcp,
            tc.tile_pool(name="qkv", bufs=1) as qkvp,
            tc.tile_pool(name="outm", bufs=1) as outp,
            tc.tile_pool(name="ps2b", bufs=2, space="PSUM") as ps2b,
            tc.tile_pool(name="ps1b", bufs=4, space="PSUM") as ps1b,
            tc.tile_pool(name="esb", bufs=4) as esbp,
            tc.tile_pool(name="ptp", bufs=2) as ptp,
            tc.tile_pool(name="bandp", bufs=2) as bandp,
            tc.tile_pool(name="smallp", bufs=4) as smallp,
            tc.tile_pool(name="ysb", bufs=3) as ysbp,
        ):
            # ---- constants / params ----
            ident0 = constp.tile([128, 128], BF16, tag="ident0")
            make_identity(nc, ident0[:])

            zrow = constp.tile([128, 137], F32, tag="zrow")
            nc.gpsimd.memset(zrow[:], 0.0)
            for i in range(2):
                for t in range(NT):
                    nc.sync.dma_start(
                        band_scr[i][t].rearrange("(p c) -> p c", c=137), zrow[:]
                    )

            wsb = {}
            for n in ("Wq", "Wk", "Wv", "Wo"):
                wsb[n] = [
                    wpool.tile([128, C], F32, tag=f"{n}{i}", name=f"{n}_{i}")
                    for i in range(CC)
                ]
                for cc in range(CC):
                    wst = xcp.tile(
                        [128, C], F32, tag="wstage", bufs=3, name=f"wst_{n}{cc}"
                    )
                    nc.sync.dma_start(
                        wst[:], w_d[n][cc * 128 : (cc + 1) * 128, :]
                    )
                    nc.vector.tensor_copy(r(wsb[n][cc][:]), wst[:])

            # per-partition bias views [128, 4]: col a = channel a*128+p
            bview = {}
            for n in ("bq", "bk", "bo"):
                t = constp.tile([128, CC], F32, tag=n)
                nc.sync.dma_start(t[:], b_d[n].rearrange("(a p) -> p a", p=128))
                bview[n] = t
            bqs = constp.tile([128, CC], F32, tag="bqs")
            nc.vector.tensor_scalar_mul(bqs[:], bview["bq"][:], SCALE)

            bv_row0 = constp.tile([1, C], F32, tag="bv_row0")
            nc.sync.dma_start(bv_row0[:], b_d["bv"][None, :])
            bv_row = constp.tile([1, C], F32, tag="bv_row")
            nc.gpsimd.tensor_copy(r(bv_row[:]), bv_row0[:])
            ones0 = constp.tile([1, 128], F32, tag="ones0")
            nc.gpsimd.memset(ones0[:], 1.0)
            ones1 = constp.tile([1, 128], F32, tag="ones1")
            nc.gpsimd.tensor_copy(r(ones1[:]), ones0[:])

            # krel as matmul rhs [64, 16] (zero-padded to 16 cols),
            # duplicated in both partition halves for odd heads
            krel0 = constp.tile([128, 16], F32, tag="krel0")
            nc.gpsimd.memset(krel0[:], 0.0)
            nc.sync.dma_start(krel0[0:KC, 0:M_REL], ek_d[0].rearrange("m k -> k m"))
            nc.sync.dma_start(krel0[KC:128, 0:M_REL], ek_d[0].rearrange("m k -> k m"))
            krel = constp.tile([128, 16], F32, tag="krel")
            nc.gpsimd.tensor_copy(r(krel[:]), krel0[:])

            vrel0 = constp.tile([M_REL, KC], F32, tag="vrel0")
            nc.sync.dma_start(vrel0[:], ev_d[0])
            vrel = constp.tile([M_REL, KC], BF16, tag="vrel")
            nc.gpsimd.tensor_copy(vrel[:], vrel0[:])

            x_sb = [
                xcp.tile([128, T], F32, tag=f"x{i}", name=f"x_sb{i}")
                for i in range(CC)
            ]
            c_sb = [
                xcp.tile([128, T], F32, tag=f"c{i}", name=f"c_sb{i}")
                for i in range(CC)
            ]
            for cc in range(CC):
                xst = xcp.tile([128, T], F32, tag="xstage", bufs=3, name=f"xst{cc}")
                nc.sync.dma_start(xst[:], x_d[cc * 128 : (cc + 1) * 128, :])
                nc.vector.tensor_copy(r(x_sb[cc][:]), xst[:])
                cst = xcp.tile([128, T], F32, tag="xstage", bufs=3, name=f"cst{cc}")
                nc.sync.dma_start(cst[:], c_d[cc * 128 : (cc + 1) * 128, :])
                nc.vector.tensor_copy(r(c_sb[cc][:]), cst[:])

            q_sb = [
                qkvp.tile([128, T], F32, tag=f"q{i}", name=f"q_sb{i}")
                for i in range(CC)
            ]
            k_sb = [
                qkvp.tile([128, T], F32, tag=f"k{i}", name=f"k_sb{i}")
                for i in range(CC)
            ]
            vt_sb = [
                qkvp.tile([128, C], BF16, tag=f"vt{i}", name=f"vt_sb{i}")
                for i in range(NT)
            ]
            out_sb = [
                outp.tile([128, T], F32, tag=f"out{i}", name=f"out_sb{i}")
                for i in range(CC)
            ]

            # ---- projections: q = (Wq^T x + bq) * scale, k = Wk^T c + bk ----
            for (dst, src, wn, bias, scale) in (
                (q_sb, x_sb, "Wq", bqs, SCALE),
                (k_sb, c_sb, "Wk", bview["bk"], 1.0),
            ):
                for dt in range(CC):
                    for tch in range(2):
                        ps = ps1b.tile([128, 512], F32, tag="p1", name=f"pj{wn}{dt}{tch}")
                        for cc in range(CC):
                            nc.tensor.matmul(
                                ps[:],
                                r(wsb[wn][cc][:, dt * 128 : (dt + 1) * 128]),
                                r(src[cc][:, tch * 512 : (tch + 1) * 512]),
                                start=(cc == 0),
                                stop=(cc == CC - 1),
                            )
                        nc.scalar.activation(
                            r(dst[dt][:, tch * 512 : (tch + 1) * 512]),
                            ps[:],
                            AF.Identity,
                            bias=bias[:, dt : dt + 1],
                            scale=scale,
                        )

            # ---- V, produced transposed in bf16:
            # vt[t, c] = sum_cc c[cc, t] Wv[cc, c] + bv
            for jt in range(NT):
                ps = ps1b.tile([128, 512], F32, tag="p1", name=f"pjv{jt}")
                for cc in range(CC):
                    nc.tensor.matmul(
                        ps[:],
                        r(c_sb[cc][:, jt * 128 : (jt + 1) * 128]),
                        r(wsb["Wv"][cc][:]),
                        start=(cc == 0),
                        stop=False,
                    )
                nc.tensor.matmul(
                    ps[:], r(ones1[:]), r(bv_row[:]), start=False, stop=True
                )
                nc.scalar.activation(vt_sb[jt][:], ps[:], AF.Copy)

            # ---- attention ----
            def head_q(h):
                return q_sb[h // 2][(h % 2) * KC : (h % 2) * KC + KC, :]

            def head_k(h):
                return k_sb[h // 2][(h % 2) * KC : (h % 2) * KC + KC, :]

            def emit_band_prep(h):
                """R9 for all 8 row tiles of head h -> skew -> banded bias
                rows Bh [128, 8*136] (f32, SBUF)."""
                qh = head_q(h)
                prow = (h % 2) * KC
                R9h = ps1b.tile([128, 128], F32, tag="p1", name=f"R9ps{h}")
                for it in range(NT):
                    nc.tensor.matmul(
                        R9h[:, it * 16 : it * 16 + 16],
                        r(qh[:, it * 128 : (it + 1) * 128]),
                        r(krel[prow : prow + KC, :]),
                        start=True,
                        stop=True,
                    )
                r9h = smallp.tile([128, 128], F32, tag="r9h", name=f"r9h{h}")
                nc.vector.tensor_copy(r9h[:], R9h[:])
                scr = band_scr[h % 2]
                nc.sync.dma_start(
                    scr.rearrange("t (p c) -> p t c", c=137)[:, :, 0:M_REL],
                    r9h[:].rearrange("p (t d) -> p t d", d=16)[:, :, 0:M_REL],
                )
                Bh = bandp.tile([128, NT * 136], F32, tag="Bh", name=f"Bh{h}")
                nc.sync.dma_start(
                    Bh[:].rearrange("p (t c) -> p t c", c=136),
                    scr[:, 0 : 128 * 136].rearrange("t (p c) -> p t c", c=136),
                )
                return Bh

            Bh_cur = emit_band_prep(0)
            for h in range(H):
                Bh_next = emit_band_prep(h + 1) if h + 1 < H else None
                qh = head_q(h)
                kh = head_k(h)
                prow = (h % 2) * KC
                headband = smallp.tile([128, NT * 136], BF16, tag="headband")
                escr = extr_scr[h % 2]
                outTs = []
                for pair in range(NT // 2):
                    i0p = pair * 256
                    pt = ptp.tile([128, 2048], BF16, tag="pt")
                    for u in range(2):
                        it = pair * 2 + u
                        i0 = i0p + u * 128

                        S = ps2b.tile([128, 1024], F32, tag="S")
                        for jch in range(2):
                            nc.tensor.matmul(
                                S[:, jch * 512 : (jch + 1) * 512],
                                r(qh[:, i0 : i0 + 128]),
                                r(kh[:, jch * 512 : (jch + 1) * 512]),
                                start=True,
                                stop=True,
                            )
                        # banded relative-key bias add (clip at edges)
                        lo = max(i0 - 4, 0)
                        hi = min(i0 + 132, T)
                        bl = lo - (i0 - 4)
                        nc.vector.tensor_add(
                            S[:, lo:hi],
                            S[:, lo:hi],
                            Bh_cur[:, it * 136 + bl : it * 136 + bl + (hi - lo)],
                        )

                        # softmax (no max subtraction; scores ~ N(0,1))
                        E = esbp.tile([128, 1032], BF16, tag="e")
                        nc.gpsimd.memset(E[:, 0:4], 0.0)
                        nc.gpsimd.memset(E[:, 1028:1032], 0.0)
                        st = smallp.tile([128, 2], F32, tag="st")
                        nc.scalar.activation(
                            E[:, 4:1028], S[:], AF.Exp, accum_out=st[:, 0:1]
                        )
                        nc.vector.reciprocal(st[:, 1:2], st[:, 0:1])
                        nc.gpsimd.tensor_scalar_mul(
                            E[:, 4:1028], E[:, 4:1028], st[:, 1:2]
                        )

                        # value-side band slice for later extraction
                        nc.gpsimd.tensor_copy(
                            headband[:, it * 136 : (it + 1) * 136],
                            E[:, i0 : i0 + 136],
                        )

                        # transpose P for the PV matmul
                        ET = ps1b.tile(
                            [128, 1024], BF16, tag="p1", name=f"ET{h}_{it}"
                        )
                        for jc in range(8):
                            nc.tensor.transpose(
                                ET[:, jc * 128 : (jc + 1) * 128],
                                E[:, 4 + jc * 128 : 4 + (jc + 1) * 128],
                                ident0[:],
                            )
                        nc.vector.tensor_copy(
                            pt[:, u * 1024 : (u + 1) * 1024], ET[:]
                        )

                    # out^T[kc, i] for the pair: PV matmuls (relative-value
                    # contribution folded in at head end)
                    outT = ps1b.tile([64, 256], F32, tag="p1", name=f"oT{h}_{pair}")
                    ptv = pt[:].rearrange("p (u n) -> p u n", u=2)
                    for jc in range(8):
                        nc.tensor.matmul(
                            outT[:],
                            vt_sb[jc][:, h * KC : (h + 1) * KC],
                            ptv[:, :, jc * 128 : (jc + 1) * 128],
                            start=(jc == 0),
                            stop=(jc == 7),
                        )
                    nc.scalar.copy(
                        r(out_sb[h // 2][prow : prow + KC, i0p : i0p + 256]),
                        outT[:],
                    )

                # head-end: inverse skew P9[p, d] = P[i0+p, i0+p+d-4] for all
                # 8 row tiles at once, then relative-value contribution
                nc.sync.dma_start(
                    escr[:, 0 : 128 * 136].rearrange("t (p c) -> p t c", c=136),
                    headband[:].rearrange("p (t c) -> p t c", c=136),
                )
                p9head = smallp.tile([128, NT * 16], BF16, tag="p9head")
                nc.sync.dma_start(
                    p9head[:].rearrange("p (t d) -> p t d", d=16)[:, :, 0:M_REL],
                    escr.rearrange("t (p c) -> p t c", c=137)[:, :, 0:M_REL],
                )
                for pair in range(NT // 2):
                    P9T = ps1b.tile(
                        [16, 256], BF16, tag="p1", name=f"P9T{h}_{pair}"
                    )
                    for u in range(2):
                        it = pair * 2 + u
                        nc.tensor.transpose(
                            P9T[0:M_REL, u * 128 : (u + 1) * 128],
                            p9head[:, it * 16 : it * 16 + M_REL],
                            ident0[:],
                        )
                    p9t = smallp.tile([16, 256], BF16, tag="p9t")
                    nc.vector.tensor_copy(p9t[0:M_REL, :], P9T[0:M_REL, :])
                    relT = ps1b.tile([64, 256], F32, tag="p1", name=f"rT{h}_{pair}")
                    nc.tensor.matmul(
                        relT[:], vrel[:], p9t[0:M_REL, :], start=True, stop=True
                    )
                    dst = out_sb[h // 2][
                        prow : prow + KC, pair * 256 : pair * 256 + 256
                    ]
                    nc.vector.tensor_add(r(dst), r(dst), relT[:])
                Bh_cur = Bh_next

            # ---- output projection: y = Wo^T out + bo ----
            for dt in range(CC):
                for tch in range(2):
                    ps = ps1b.tile([128, 512], F32, tag="p1", name=f"pyo{dt}{tch}")
                    for cc in range(CC):
                        nc.tensor.matmul(
                            ps[:],
                            r(wsb["Wo"][cc][:, dt * 128 : (dt + 1) * 128]),
                            r(out_sb[cc][:, tch * 512 : (tch + 1) * 512]),
                            start=(cc == 0),
                            stop=(cc == CC - 1),
                        )
                    yt = ysbp.tile([128, 512], F32, tag="y")
                    nc.scalar.activation(
                        yt[:],
                        ps[:],
                        AF.Identity,
                        bias=bview["bo"][:, dt : dt + 1],
                        scale=1.0,
                    )
                    nc.sync.dma_start(
                        y_d[dt * 128 : (dt + 1) * 128, tch * 512 : (tch + 1) * 512],
                        yt[:],
                    )

    nc.compile()
    return nc


_NC_CACHE = None


def kernel(x, c, Wq, bq, Wk, bk, Wv, bv, Wo, bo, emb_rel_k, emb_rel_v):
    global _NC_CACHE
    if _NC_CACHE is None:
        _NC_CACHE = build_kernel()
    nc = _NC_CACHE

    def f32(a):
        return np.ascontiguousarray(np.asarray(a), dtype=np.float32)

    shared = {
        "Wq": f32(Wq), "bq": f32(bq), "Wk": f32(Wk), "bk": f32(bk),
        "Wv": f32(Wv), "bv": f32(bv), "Wo": f32(Wo), "bo": f32(bo),
        "emb_rel_k": f32(emb_rel_k), "emb_rel_v": f32(emb_rel_v),
    }
    in_maps = [
        {"x": f32(x[b]), "c": f32(c[b]), **shared} for b in range(N_CORES)
    ]
    res = run_bass_kernel_spmd(nc, in_maps, core_ids=list(range(N_CORES)))
    return np.stack([res.results[b]["y"] for b in range(N_CORES)], axis=0)


# revision 22
# speedup vs baseline: 1.1014x; 1.1014x over previous
"""Trainium2 Bass kernel for VITS-style multi-head attention with windowed
relative position embeddings (window=4), batch 8 x channels 512 x time 1024.

Strategy: pure data parallelism — one batch element per NeuronCore (8 cores).
Per core:
  - Q/K projections in [channel, time] layout (float32r); V projected directly
    transposed as [time, channel] in bf16 (PV stationary operand).
  - Per head: the relative-key logits R9 = Qs @ Krel^T [T, 9] are computed for
    all 8 row-tiles up-front (prefetched one head ahead) and skewed into
    banded [128, 8*136] bias rows via one stride-137-write / stride-136-read
    DRAM round trip (writing row p at pitch 137 and reading at pitch 136
    shifts row p right by p).
  - Per 128-row tile: scores = Qs K^T via PE (f32r), banded bias added in
    PSUM, exp (ACT, free-dim accumulate, no max subtraction — scores ~N(0,1)),
    normalize (DVE, bf16 out), PE-transpose P (bf16) for the PV matmul.
  - Value-side band P9[i,d] = P[i, i+d-4] extracted per pair of row tiles via
    the inverse skew (write pitch 136, read pitch 137) and folded into the
    output PSUM as a [9,64] matmul alongside PV.
"""

import os
import sys

sys.path.insert(0, "/opt/trn_rl_repo")

import numpy as np

import concourse.bass as bass
import concourse.mybir as mybir
import concourse.tile as tile
from concourse import bacc
from concourse.bass_utils import run_bass_kernel_spmd
from concourse.masks import make_identity

B, C, T = 8, 512, 1024
H = 8
KC = C // H  # 64
WINDOW = 4
M_REL = 2 * WINDOW + 1  # 9
SCALE = 1.0 / float(np.sqrt(KC))
N_CORES = 8
NT = T // 128  # 8 row tiles per head
SEG = 128 * 137  # per-row-tile skew scratch segment (elements)

F32 = mybir.dt.float32
F32R = mybir.dt.float32r
BF16 = mybir.dt.bfloat16
AF = mybir.ActivationFunctionType


def r(ap):
    return ap.bitcast(F32R)


def build_kernel():
    nc = bacc.Bacc(
        "TRN2", target_bir_lowering=False, debug=False, num_devices=N_CORES
    )

    x_d = nc.dram_tensor("x", [C, T], F32, kind="ExternalInput")
    c_d = nc.dram_tensor("c", [C, T], F32, kind="ExternalInput")
    w_d = {
        n: nc.dram_tensor(n, [C, C], F32, kind="ExternalInput")
        for n in ("Wq", "Wk", "Wv", "Wo")
    }
    b_d = {
        n: nc.dram_tensor(n, [C], F32, kind="ExternalInput")
        for n in ("bq", "bk", "bv", "bo")
    }
    ek_d = nc.dram_tensor("emb_rel_k", [1, M_REL, KC], F32, kind="ExternalInput")
    ev_d = nc.dram_tensor("emb_rel_v", [1, M_REL, KC], F32, kind="ExternalInput")
    y_d = nc.dram_tensor("y", [C, T], F32, kind="ExternalOutput")

    # DRAM skew scratches, one 128x137-element segment per row tile.
    # band (f32): write [128,9] at pitch 137, read [128,136] at pitch 136
    # (gaps must stay zero). extr (bf16): write [128,136] at pitch 136, read
    # [128,9] at pitch 137.
    band_scr = [
        nc.dram_tensor(f"band_scr{i}", [NT, SEG], F32, kind="Internal")
        for i in range(2)
    ]
    extr_scr = [
        nc.dram_tensor(f"extr_scr{i}", [NT, SEG], BF16, kind="Internal")
        for i in range(2)
    ]

    CC = C // 128  # 4 channel chunks

    with tile.TileContext(nc) as tc:
        with (
            tc.tile_pool(name="const", bufs=1) as constp,
            tc.tile_pool(name="wo", bufs=1) as wop,
            tc.tile_pool(name="qkv", bufs=1) as qkvp,
            tc.tile_pool(name="outm", bufs=1) as outp,
            tc.tile_pool(name="ps2b", bufs=2, space="PSUM") as ps2b,
            tc.tile_pool(name="psET", bufs=2, space="PSUM") as psET,
            tc.tile_pool(name="ps1b", bufs=2, space="PSUM") as ps1b,
            tc.tile_pool(name="esb", bufs=6) as esbp,
            tc.tile_pool(name="ptp", bufs=3) as ptp,
            tc.tile_pool(name="bandp", bufs=4) as bandp,
            tc.tile_pool(name="smallp", bufs=2) as smallp,
            tc.tile_pool(name="ysb", bufs=3) as ysbp,
        ):
            # ---- constants / params ----
            ident0 = constp.tile([128, 128], BF16, tag="ident0")
            make_identity(nc, ident0[:])

            projpool = tc.tile_pool(name="projtmp", bufs=1)
            xcp = projpool.__enter__()

            # per-partition bias views [128, 4]: col a = channel a*128+p
            bview = {}
            for n in ("bq", "bk", "bo"):
                t = constp.tile([128, CC], F32, tag=n)
                nc.sync.dma_start(t[:], b_d[n].rearrange("(a p) -> p a", p=128))
                bview[n] = t
            bqs = constp.tile([128, CC], F32, tag="bqs")
            nc.vector.tensor_scalar_mul(bqs[:], bview["bq"][:], SCALE)

            bv_row0 = constp.tile([1, C], F32, tag="bv_row0")
            nc.sync.dma_start(bv_row0[:], b_d["bv"][None, :])
            bv_row = constp.tile([1, C], F32, tag="bv_row")
            nc.gpsimd.tensor_copy(r(bv_row[:]), bv_row0[:])
            ones0 = constp.tile([1, 128], F32, tag="ones0")
            nc.gpsimd.memset(ones0[:], 1.0)
            ones1 = constp.tile([1, 128], F32, tag="ones1")
            nc.gpsimd.tensor_copy(r(ones1[:]), ones0[:])

            # krel as matmul rhs [64, 16] (zero-padded to 16 cols),
            # duplicated in both partition halves for odd heads
            krel0 = constp.tile([128, 16], F32, tag="krel0")
            nc.gpsimd.memset(krel0[:], 0.0)
            nc.sync.dma_start(krel0[0:KC, 0:M_REL], ek_d[0].rearrange("m k -> k m"))
            nc.sync.dma_start(krel0[KC:128, 0:M_REL], ek_d[0].rearrange("m k -> k m"))
            krel = constp.tile([128, 16], F32, tag="krel")
            nc.gpsimd.tensor_copy(r(krel[:]), krel0[:])

            vrel0 = constp.tile([M_REL, KC], F32, tag="vrel0")
            nc.sync.dma_start(vrel0[:], ev_d[0])
            vrel = constp.tile([M_REL, KC], BF16, tag="vrel")
            nc.gpsimd.tensor_copy(vrel[:], vrel0[:])

            x_sb = [
                xcp.tile([128, T], F32, tag=f"x{i}", name=f"x_sb{i}")
                for i in range(CC)
            ]
            c_sb = [
                xcp.tile([128, T], F32, tag=f"c{i}", name=f"c_sb{i}")
                for i in range(CC)
            ]
            for cc in range(CC):
                for tch in range(2):
                    cs = slice(tch * 512, (tch + 1) * 512)
                    xst = xcp.tile(
                        [128, 512], F32, tag="wstage", bufs=4,
                        name=f"xst{cc}_{tch}"
                    )
                    nc.sync.dma_start(xst[:], x_d[cc * 128 : (cc + 1) * 128, cs])
                    nc.vector.tensor_copy(r(x_sb[cc][:, cs]), xst[:])
                    cst = xcp.tile(
                        [128, 512], F32, tag="wstage", bufs=4,
                        name=f"cst{cc}_{tch}"
                    )
                    nc.sync.dma_start(cst[:], c_d[cc * 128 : (cc + 1) * 128, cs])
                    nc.gpsimd.tensor_copy(r(c_sb[cc][:, cs]), cst[:])

            wsb = {}
            for n in ("Wq", "Wk", "Wv", "Wo"):
                pool_n = wop if n == "Wo" else xcp
                wsb[n] = [
                    pool_n.tile([128, C], F32, tag=f"{n}{i}", name=f"{n}_{i}")
                    for i in range(CC)
                ]
                for cc in range(CC):
                    wst = xcp.tile(
                        [128, 512], F32, tag="wstage", bufs=4, name=f"wst_{n}{cc}"
                    )
                    nc.sync.dma_start(
                        wst[:], w_d[n][cc * 128 : (cc + 1) * 128, :]
                    )
                    if n in ("Wq", "Wv"):
                        nc.vector.tensor_copy(r(wsb[n][cc][:]), wst[:])
                    else:
                        nc.gpsimd.tensor_copy(r(wsb[n][cc][:]), wst[:])

            zrow = constp.tile([128, 137], F32, tag="zrow")
            nc.gpsimd.memset(zrow[:], 0.0)
            for i in range(2):
                for t in range(NT):
                    nc.sync.dma_start(
                        band_scr[i][t].rearrange("(p c) -> p c", c=137), zrow[:]
                    )

            q_sb = [
                qkvp.tile([128, T], F32, tag=f"q{i}", name=f"q_sb{i}")
                for i in range(CC)
            ]
            k_sb = [
                qkvp.tile([128, T], F32, tag=f"k{i}", name=f"k_sb{i}")
                for i in range(CC)
            ]
            vt_sb = [
                qkvp.tile([128, C], BF16, tag=f"vt{i}", name=f"vt_sb{i}")
                for i in range(NT)
            ]
            out_sb = [
                outp.tile([128, T], F32, tag=f"out{i}", name=f"out_sb{i}")
                for i in range(CC)
            ]

            # ---- projections: q = (Wq^T x + bq) * scale, k = Wk^T c + bk.
            # Emit the d-tiles heads 0/1 need first, then V (needed by the
            # first PV matmuls), then the rest, so attention starts early.
            def proj_order():
                for dt in range(CC):
                    yield (q_sb, x_sb, "Wq", bqs, SCALE, dt)
                    yield (k_sb, c_sb, "Wk", bview["bk"], 1.0, dt)
                    if dt == 0:
                        yield None  # V marker
            for item in proj_order():
                if item is None:
                    for jt in range(NT):
                        ps = psET.tile([128, 512], F32, tag="ET", name=f"pjv{jt}")
                        for cc in range(CC):
                            nc.tensor.matmul(
                                ps[:],
                                r(c_sb[cc][:, jt * 128 : (jt + 1) * 128]),
                                r(wsb["Wv"][cc][:]),
                                start=(cc == 0),
                                stop=False,
                            )
                        nc.tensor.matmul(
                            ps[:], r(ones1[:]), r(bv_row[:]), start=False,
                            stop=True,
                        )
                        nc.scalar.activation(vt_sb[jt][:], ps[:], AF.Copy)
                    continue
                (dst, src, wn, bias, scale, dt) = item
                if True:
                    for tch in range(2):
                        ps = psET.tile([128, 512], F32, tag="ET", name=f"pj{wn}{dt}{tch}")
                        for cc in range(CC):
                            nc.tensor.matmul(
                                ps[:],
                                r(wsb[wn][cc][:, dt * 128 : (dt + 1) * 128]),
                                r(src[cc][:, tch * 512 : (tch + 1) * 512]),
                                start=(cc == 0),
                                stop=(cc == CC - 1),
                            )
                        nc.scalar.activation(
                            r(dst[dt][:, tch * 512 : (tch + 1) * 512]),
                            ps[:],
                            AF.Identity,
                            bias=bias[:, dt : dt + 1],
                            scale=scale,
                        )

            projpool.__exit__(None, None, None)

            # ---- attention ----
            def head_q(h):
                return q_sb[h // 2][(h % 2) * KC : (h % 2) * KC + KC, :]

            def head_k(h):
                return k_sb[h // 2][(h % 2) * KC : (h % 2) * KC + KC, :]

            def emit_band_prep(h):
                """R9 for all 8 row tiles of head h -> skew -> banded bias
                rows Bh [128, 8*136] (f32, SBUF)."""
                qh = head_q(h)
                prow = (h % 2) * KC
                R9h = ps1b.tile([128, 128], F32, tag="p1", name=f"R9ps{h}")
                for it in range(NT):
                    nc.tensor.matmul(
                        R9h[:, it * 16 : it * 16 + 16],
                        r(qh[:, it * 128 : (it + 1) * 128]),
                        r(krel[prow : prow + KC, :]),
                        start=True,
                        stop=True,
                    )
                r9h = smallp.tile([128, 128], F32, tag="r9h", bufs=4, name=f"r9h{h}")
                nc.vector.tensor_copy(r9h[:], R9h[:])
                scr = band_scr[h % 2]
                nc.sync.dma_start(
                    scr.rearrange("t (p c) -> p t c", c=137)[:, :, 0:M_REL],
                    r9h[:].rearrange("p (t d) -> p t d", d=16)[:, :, 0:M_REL],
                )
                Bh = bandp.tile([128, NT * 136], F32, tag="Bh", name=f"Bh{h}")
                nc.sync.dma_start(
                    Bh[:].rearrange("p (t c) -> p t c", c=136),
                    scr[:, 0 : 128 * 136].rearrange("t (p c) -> p t c", c=136),
                )
                return Bh

            def emit_pair(h, pair, Bh):
                qh = head_q(h)
                kh = head_k(h)
                prow = (h % 2) * KC
                i0p = pair * 256
                pt = ptp.tile([128, 2048], BF16, tag="pt", name=f"pt{h}_{pair}")
                pairband = smallp.tile(
                    [128, 2 * 136], BF16, tag="pairband", bufs=4,
                    name=f"pb{h}_{pair}"
                )
                # distinct scratch segments per (head parity, pair parity)
                escr = extr_scr[h % 2]
                seg0 = (pair % 2) * 2
                for u in range(2):
                    it = pair * 2 + u
                    i0 = i0p + u * 128

                    S = ps2b.tile([128, 1024], F32, tag="S", name=f"S{h}_{it}")
                    for jch in range(2):
                        nc.tensor.matmul(
                            S[:, jch * 512 : (jch + 1) * 512],
                            r(qh[:, i0 : i0 + 128]),
                            r(kh[:, jch * 512 : (jch + 1) * 512]),
                            start=True,
                            stop=True,
                        )
                    # banded relative-key bias add (clip at edges)
                    lo = max(i0 - 4, 0)
                    hi = min(i0 + 132, T)
                    bl = lo - (i0 - 4)
                    nc.vector.tensor_add(
                        S[:, lo:hi],
                        S[:, lo:hi],
                        Bh[:, it * 136 + bl : it * 136 + bl + (hi - lo)],
                    )

                    # softmax (no max subtraction; scores ~ N(0,1))
                    E = esbp.tile([128, 1032], BF16, tag="e", name=f"E{h}_{it}")
                    nc.gpsimd.memset(E[:, 0:4], 0.0)
                    nc.gpsimd.memset(E[:, 1028:1032], 0.0)
                    st = smallp.tile(
                        [128, 2], F32, tag="st", bufs=8, name=f"st{h}_{it}"
                    )
                    nc.scalar.activation(
                        E[:, 4:1028], S[:], AF.Exp, accum_out=st[:, 0:1]
                    )
                    nc.vector.reciprocal(st[:, 1:2], st[:, 0:1])
                    nc.vector.tensor_scalar_mul(
                        E[:, 4:1028], E[:, 4:1028], st[:, 1:2]
                    )

                    # value-side band slice for later extraction
                    nc.gpsimd.tensor_copy(
                        pairband[:, u * 136 : (u + 1) * 136],
                        E[:, i0 : i0 + 136],
                    )

                    # transpose P for the PV matmul
                    ET = psET.tile(
                        [128, 1024], BF16, tag="ET", name=f"ET{h}_{it}"
                    )
                    for jc in range(8):
                        nc.tensor.transpose(
                            ET[:, jc * 128 : (jc + 1) * 128],
                            E[:, 4 + jc * 128 : 4 + (jc + 1) * 128],
                            ident0[:],
                        )
                    nc.vector.tensor_copy(
                        pt[:, u * 1024 : (u + 1) * 1024], ET[:]
                    )

                # inverse skew: P9[p, d] = P[i0+p, i0+p+d-4] per sub-tile
                nc.sync.dma_start(
                    escr[seg0 : seg0 + 2, 0 : 128 * 136].rearrange(
                        "t (p c) -> p t c", c=136
                    ),
                    pairband[:].rearrange("p (t c) -> p t c", c=136),
                )
                p9pair = smallp.tile(
                    [128, 32], BF16, tag="p9pair", bufs=4, name=f"p9p{h}_{pair}"
                )
                nc.sync.dma_start(
                    p9pair[:].rearrange("p (t d) -> p t d", d=16)[:, :, 0:M_REL],
                    escr[seg0 : seg0 + 2].rearrange("t (p c) -> p t c", c=137)[
                        :, :, 0:M_REL
                    ],
                )
                P9T = ps1b.tile([16, 256], BF16, tag="p1", name=f"P9T{h}_{pair}")
                for u in range(2):
                    nc.tensor.transpose(
                        P9T[0:M_REL, u * 128 : (u + 1) * 128],
                        p9pair[:, u * 16 : u * 16 + M_REL],
                        ident0[:],
                    )
                p9t = smallp.tile(
                    [16, 256], BF16, tag="p9t", bufs=4, name=f"p9t{h}_{pair}"
                )
                nc.vector.tensor_copy(p9t[0:M_REL, :], P9T[0:M_REL, :])

                # out^T[kc, i] for the pair: PV matmuls; the relative-value
                # term goes to its own PSUM tile and is added afterwards so
                # the out copy never waits on the extraction round trip
                outT = ps1b.tile([64, 256], F32, tag="p1", name=f"oT{h}_{pair}")
                ptv = pt[:].rearrange("p (u n) -> p u n", u=2)
                for jc in range(8):
                    nc.tensor.matmul(
                        outT[:],
                        vt_sb[jc][:, h * KC : (h + 1) * KC],
                        ptv[:, :, jc * 128 : (jc + 1) * 128],
                        start=(jc == 0),
                        stop=(jc == 7),
                    )
                dst = out_sb[h // 2][prow : prow + KC, i0p : i0p + 256]
                nc.scalar.copy(r(dst), outT[:])
                relT = ps1b.tile([64, 256], F32, tag="p1", name=f"rT{h}_{pair}")
                nc.tensor.matmul(
                    relT[:], vrel[:], p9t[0:M_REL, :], start=True, stop=True
                )
                nc.vector.tensor_add(r(dst), r(dst), relT[:])

            # two heads interleaved for deeper independent pipelines; band
            # prep prefetched one head-pair ahead
            Bh = {0: emit_band_prep(0), 1: emit_band_prep(1)}
            for hp in range(H // 2):
                hA, hB = 2 * hp, 2 * hp + 1
                if hp + 1 < H // 2:
                    Bh[hA + 2] = emit_band_prep(hA + 2)
                    Bh[hB + 2] = emit_band_prep(hB + 2)
                for pair in range(NT // 2):
                    emit_pair(hA, pair, Bh[hA])
                    emit_pair(hB, pair, Bh[hB])

            # ---- output projection: y = Wo^T out + bo ----
            for dt in range(CC):
                for tch in range(2):
                    ps = psET.tile([128, 512], F32, tag="ET", name=f"pyo{dt}{tch}")
                    for cc in range(CC):
                        nc.tensor.matmul(
                            ps[:],
                            r(wsb["Wo"][cc][:, dt * 128 : (dt + 1) * 128]),
                            r(out_sb[cc][:, tch * 512 : (tch + 1) * 512]),
                            start=(cc == 0),
                            stop=(cc == CC - 1),
                        )
                    yt = ysbp.tile([128, 512], F32, tag="y")
                    nc.scalar.activation(
                        yt[:],
                        ps[:],
                        AF.Identity,
                        bias=bview["bo"][:, dt : dt + 1],
                        scale=1.0,
                    )
                    nc.sync.dma_start(
                        y_d[dt * 128 : (dt + 1) * 128, tch * 512 : (tch + 1) * 512],
                        yt[:],
                    )

    nc.compile()
    return nc


_NC_CACHE = None


def kernel(x, c, Wq, bq, Wk, bk, Wv, bv, Wo, bo, emb_rel_k, emb_rel_v):
    global _NC_CACHE
    if _NC_CACHE is None:
        _NC_CACHE = build_kernel()
    nc = _NC_CACHE

    def f32(a):
        return np.ascontiguousarray(np.asarray(a), dtype=np.float32)

    shared = {
        "Wq": f32(Wq), "bq": f32(bq), "Wk": f32(Wk), "bk": f32(bk),
        "Wv": f32(Wv), "bv": f32(bv), "Wo": f32(Wo), "bo": f32(bo),
        "emb_rel_k": f32(emb_rel_k), "emb_rel_v": f32(emb_rel_v),
    }
    in_maps = [
        {"x": f32(x[b]), "c": f32(c[b]), **shared} for b in range(N_CORES)
    ]
    res = run_bass_kernel_spmd(nc, in_maps, core_ids=list(range(N_CORES)))
    return np.stack([res.results[b]["y"] for b in range(N_CORES)], axis=0)
